# revision 2
# baseline (speedup 1.0000x reference)
"""Guided filter, 8 TRN2 cores, 1 image/core — all-PE box filters.

Each 31x31 box = two PE "Type-2" passes (data stationary in bf16, bf16 0/1
band matrix moving): out[m,n] = sum_k src[k,m]*band[k,n] box-filters along
the partition dim AND transposes, so two passes restore orientation. Band
sparsity -> 4 overlapping column-range pieces per output PSUM bank (zero-
region semantics permit overlapping accumulation after one start=True).
A PSUM->SBUF bf16 "mid" copy sits between the passes (PE reads SBUF only).

Stats are uncentered; the var/cov mean corrections (vs = sg^2, u2 =
mpp*sg) are computed in f32 and subtracted IN PSUM via -I @ x f32r
identity matmuls joining the pass-2 accumulation group. bf16 lives only
on box inputs/mids where rounding is random and averages out.

  s1 = g0*(WR/WB)+g2 ; s2 = g1*(WG/WB)+s1  (= gray/WB, kept for w)
  G = s2*WB ; G2 = G*G ; P_c = p_c ; GP_c = G*P_c            [bf16]
  ps_G = box(G) ; sg = ps_G/31
  ps_G2 = box(G2) - I@(sg^2)        -> rr = 1/ps_G2 (= 1/(961 var))
  ps_P = box(P_c) ; mpp = ps_P/31
  ps_GP = box(GP_c) - I@(mpp*sg)    -> covR in PSUM
  a = ps_GP*rr ; vpr = ps_GP*(rr*sg) ; bR = mpp-vpr          [bf16]
  ps_B = box(bR) + I@wq ; wq = (box(a)*K2*WB*29791)*s2
  out = ps_B * K2/31
"""
import sys

sys.path.insert(0, "/opt/trn_rl_repo")

import numpy as np
import ml_dtypes
import concourse.bass as bass
import concourse.bacc as bacc
import concourse.mybir as mybir
import concourse.tile as tile
from concourse import bass_utils
from contextlib import ExitStack

F32 = mybir.dt.float32
F32R = mybir.dt.float32r
BF16 = mybir.dt.bfloat16
ALU = mybir.AluOpType
ACT = mybir.ActivationFunctionType

R = 15
H = W = 512
NB = 4
WR, WG, WB = 0.299, 0.587, 0.114
S31 = 1.0 / 31.0
K2 = 1.0 / 961.0
FUSE = 961.0 * 31.0


def band_np():
    """BND[p, b, n] = 1 if |128b+p - n| <= R else 0."""
    k = np.arange(512)
    B = (np.abs(k[:, None] - k[None, :]) <= R).astype(np.float32)
    return B.reshape(4, 128, 512).transpose(1, 0, 2).copy()


def _build(nc):
    guide_d = nc.dram_tensor("guide", [3, H, W], F32R, kind="ExternalInput").ap()
    lum_d = nc.dram_tensor("lum", [128, 3, 128], F32R, kind="ExternalInput").ap()
    input_d = nc.dram_tensor("input", [3, H, W], F32, kind="ExternalInput").ap()
    bnd_d = nc.dram_tensor("bnd", [128, NB, W], BF16, kind="ExternalInput").ap()
    eye_d = nc.dram_tensor("eye", [128, 128], BF16, kind="ExternalInput").ap()
    ney_d = nc.dram_tensor("ney", [128, 128], F32R, kind="ExternalInput").ap()
    out_d = nc.dram_tensor("out", [3, H, W], F32, kind="ExternalOutput").ap()

    def plane(dram, c):
        return dram[c].rearrange("(b p) w -> p b w", p=128)

    def plane_half(dram, c, h):
        return dram[c].rearrange("(b p) w -> p b w", p=128)[:, 2 * h:2 * h + 2, :]

    with tile.TileContext(nc) as tc, ExitStack() as ctx:
        cons = ctx.enter_context(tc.tile_pool(name="cons", bufs=1))
        gpool = ctx.enter_context(tc.tile_pool(name="gpool", bufs=1))
        ppool = ctx.enter_context(tc.tile_pool(name="ppool", bufs=1))
        bfp = ctx.enter_context(tc.tile_pool(name="bfp", bufs=1))
        midp = ctx.enter_context(tc.tile_pool(name="midp", bufs=4))
        stat = ctx.enter_context(tc.tile_pool(name="stat", bufs=1))
        ps1p = ctx.enter_context(tc.tile_pool(name="ps1p", bufs=1, space="PSUM"))
        ps2p = ctx.enter_context(tc.tile_pool(name="ps2p", bufs=2, space="PSUM"))

        BND = cons.tile([128, NB, W], BF16, tag="BND", name="BND")
        nc.sync.dma_start(BND[:], bnd_d)
        EYE = cons.tile([128, 128], BF16, tag="EYE", name="EYE")
        nc.sync.dma_start(EYE[:], eye_d)
        NEY = cons.tile([128, 128], F32R, tag="NEY", name="NEY")
        nc.sync.dma_start(NEY[:], ney_d)

        g = [gpool.tile([128, NB, W], F32R, tag=f"g{c}", name=f"g{c}")
             for c in range(3)]
        LUM = cons.tile([128, 3, 128], F32R, tag="LUM", name="LUM")
        nc.sync.dma_start(LUM[:], lum_d)
        p = [ppool.tile([128, NB, W], F32, tag=f"p{c}", name=f"p{c}")
             for c in range(3)]
        # guide planes in halves; p0 early so box(P0) leads the PE stream
        for c in (0, 1, 2):
            nc.sync.dma_start(g[c][:, 0:2, :], plane_half(guide_d, c, 0))
        nc.sync.dma_start(p[0][:], plane(input_d, 0))
        for c in (0, 1, 2):
            nc.sync.dma_start(g[c][:, 2:4, :], plane_half(guide_d, c, 1))
        nc.sync.dma_start(p[1][:], plane(input_d, 1))
        nc.sync.dma_start(p[2][:], plane(input_d, 2))

        PCS = []
        for ki in range(NB):
            PCS.append((ki, max(0, 128 * ki - R), min(512, 128 * ki + 128 + R)))

        def pass1(dst_ps, src):
            for kgrp in ((0, 1), (2, 3)):
                for mi in range(NB):
                    for ki in kgrp:
                        _, n0, n1 = PCS[ki]
                        nc.tensor.matmul(
                            dst_ps[:, mi, n0:n1],
                            src[:, ki, 128 * mi:128 * mi + 128],
                            BND[:, ki, n0:n1],
                            start=(ki == 0), stop=(ki == NB - 1))

        def pass2_half(dst, ymid, h, fuse):
            """fuse: None | ('bf16', rhs_tile) | ('f32r', rhs_tile)"""
            for j in range(2):
                mi = 2 * h + j
                for ki in range(NB):
                    _, n0, n1 = PCS[ki]
                    nc.tensor.matmul(
                        dst[:, j, n0:n1],
                        ymid[:, ki, 128 * mi:128 * mi + 128],
                        BND[:, ki, n0:n1],
                        start=(ki == 0),
                        stop=(ki == NB - 1 and fuse is None))
                if fuse is not None:
                    kind, rhs = fuse
                    if kind == "bf16":
                        nc.tensor.matmul(
                            dst[:, j, :], EYE[:], rhs[:, mi, :],
                            start=False, stop=True)
                    else:
                        nc.tensor.matmul(
                            dst[:, j, :], NEY[:], rhs[:, mi, :],
                            start=False, stop=True)

        def box(src, mid_engine, name, fuse=None):
            """[ps_half0, ps_half1] raw 31x31 box sums of src (+fused adds)."""
            ps1 = ps1p.tile([128, NB, W], F32, tag="ps1", name=f"ps1_{name}")
            pass1(ps1, src)
            ymid = midp.tile([128, NB, W], BF16, tag="mid", name=f"mid_{name}")
            if mid_engine == "act":
                nc.scalar.copy(ymid[:], ps1[:])
            else:
                nc.vector.tensor_copy(ymid[:], ps1[:])
            halves = []
            for h in range(2):
                ph = ps2p.tile([128, 2, W], F32, tag="ps2", name=f"ps2_{name}{h}")
                pass2_half(ph, ymid, h, fuse)
                halves.append(ph)
            return halves

        HV = [slice(0, 2), slice(2, 4)]

        # ---- luma on PE: ps_lum[b] = sum_c (w_c/WB * I) @ g_c[b] ----
        G = bfp.tile([128, NB, W], BF16, tag="G", name="G")
        for h in range(2):
            ps_lum = ps2p.tile([128, 2, W], F32, tag="ps2", name=f"ps_lum{h}")
            for j in range(2):
                b = 2 * h + j
                for ci in range(3):
                    nc.tensor.matmul(ps_lum[:, j, :], LUM[:, ci, :],
                                     g[ci][:, b, :], start=(ci == 0),
                                     stop=(ci == 2))
            if h == 0:
                nc.scalar.copy(G[:, 0:2, :], ps_lum[:])
            else:
                nc.vector.tensor_copy(G[:, 2:4, :], ps_lum[:])
        G2 = bfp.tile([128, NB, W], BF16, tag="G2", name="G2")
        for h in range(2):
            nc.gpsimd.tensor_mul(G2[:, HV[h], :], G[:, HV[h], :],
                                 G[:, HV[h], :])

        Pb = [None, None, None]
        mpp = [None, None, None]

        def p_cluster(c, mid_engine):
            Pb[c] = bfp.tile([128, NB, W], BF16, tag=f"Pb{c}", name=f"Pb{c}")
            nc.vector.tensor_copy(Pb[c][:], p[c][:])
            hP = box(Pb[c], mid_engine, f"P{c}")
            mpp[c] = stat.tile([128, NB, W], F32, tag=f"mpp{c}",
                               name=f"mpp{c}")
            for h in range(2):
                nc.scalar.mul(mpp[c][:, HV[h], :], hP[h][:], S31)

        p_cluster(0, "dve")

        # ---- box(G): sg + vs (vs = (ps*S31)^2 straight from PSUM, f32r)
        hG = box(G, "act", "G")
        sg = stat.tile([128, NB, W], F32, tag="sg", name="sg")
        vs = stat.tile([128, NB, W], F32R, tag="vs", name="vs")
        for h in range(2):
            nc.scalar.mul(sg[:, HV[h], :], hG[h][:], S31)
            nc.scalar.activation(vs[:, HV[h], :], hG[h][:], ACT.Square,
                                 scale=S31)
        sg_bf = stat.tile([128, NB, W], BF16, tag="sg_bf", name="sg_bf")
        nc.vector.tensor_copy(sg_bf[:], sg[:])

        p_cluster(1, "act")

        # ---- box(G2) - I@vs -> rr ----
        hG2 = box(G2, "dve", "G2", fuse=("f32r", vs))
        rr = stat.tile([128, NB, W], F32, tag="rr", name="rr")
        for h in range(2):
            nc.vector.reciprocal_approx_fast(rr[:, HV[h], :], hG2[h][:])
        rr_bf = stat.tile([128, NB, W], BF16, tag="rr_bf", name="rr_bf")
        nc.vector.tensor_copy(rr_bf[:], rr[:])
        t1 = stat.tile([128, NB, W], BF16, tag="t1", name="t1")
        nc.vector.tensor_tensor(t1[:], rr_bf[:], sg_bf[:], ALU.mult)

        p_cluster(2, "act")

        # ---- per-channel covariance chains ----
        cv = [None, None, None]
        a_t = [None, None, None]
        bR = [None, None, None]
        for c, mid_e in ((0, "dve"), (1, "act"), (2, "dve")):
            GPt = bfp.tile([128, NB, W], BF16, tag=f"GP{c}", name=f"GP{c}")
            nc.gpsimd.tensor_mul(GPt[:], G[:], Pb[c][:])
            u2 = stat.tile([128, NB, W], F32R, tag="scr", name=f"u2{c}")
            for h in range(2):
                nc.gpsimd.tensor_mul(u2[:, HV[h], :], mpp[c][:, HV[h], :],
                                     sg[:, HV[h], :])
            hGP = box(GPt, mid_e, f"GP{c}", fuse=("f32r", u2))
            cv[c] = stat.tile([128, NB, W], BF16, tag=f"cv{c}", name=f"cv{c}")
            for h in range(2):
                nc.scalar.copy(cv[c][:, HV[h], :], hGP[h][:])
            a_t[c] = bfp.tile([128, NB, W], BF16, tag=f"Pb{c}", name=f"a{c}")
            nc.vector.tensor_tensor(a_t[c][:], cv[c][:], rr_bf[:], ALU.mult)
            vpr = bfp.tile([128, NB, W], BF16, tag=f"GP{c}", name=f"vpr{c}")
            nc.vector.tensor_tensor(vpr[:], cv[c][:], t1[:], ALU.mult)
            bR[c] = stat.tile([128, NB, W], BF16, tag=f"bR{c}", name=f"bR{c}")
            nc.vector.tensor_tensor(bR[c][:], mpp[c][:], vpr[:], ALU.subtract)

        # ---- output boxes, staggered A0 A1 B0 A2 B1 B2 ----
        wq = [None, None, None]

        def a_cluster(c, mid_e):
            hA = box(a_t[c], mid_e, f"A{c}")
            wq[c] = gpool.tile([128, NB, W], BF16, tag=f"g{c}", name=f"wq{c}")
            for h in range(2):
                nc.vector.scalar_tensor_tensor(
                    wq[c][:, HV[h], :], hA[h][:], 31.0,
                    G[:, HV[h], :], ALU.mult, ALU.mult)

        def b_cluster(c, mid_e):
            hB = box(bR[c], mid_e, f"B{c}", fuse=("bf16", wq[c]))
            ot = ppool.tile([128, NB, W], F32, tag=f"p{c}", name=f"ot{c}")
            for h in range(2):
                nc.scalar.mul(ot[:, HV[h], :], hB[h][:], K2 * S31)
            nc.sync.dma_start(plane(out_d, c), ot[:])

        a_cluster(0, "act")
        a_cluster(1, "dve")
        b_cluster(0, "act")
        a_cluster(2, "dve")
        b_cluster(1, "act")
        b_cluster(2, "dve")

    nc.compile()
    return nc


_NC_CACHE = None


def _get_nc():
    global _NC_CACHE
    if _NC_CACHE is None:
        nc = bacc.Bacc("TRN2", target_bir_lowering=False, debug=False)
        _build(nc)
        _NC_CACHE = nc
    return _NC_CACHE


_CONSTS = None


def _get_consts():
    global _CONSTS
    if _CONSTS is None:
        lum = np.stack([np.eye(128, dtype=np.float32) * (w / WB)
                        for w in (WR, WG, WB)])   # [3,128,128]
        lum = lum.transpose(1, 0, 2).copy()       # [128,3,128] lhsT layout
        _CONSTS = (band_np().astype(ml_dtypes.bfloat16),
                   np.eye(128, dtype=np.float32).astype(ml_dtypes.bfloat16),
                   (-np.eye(128, dtype=np.float32)), lum)
    return _CONSTS


def kernel(**inputs):
    guide = np.ascontiguousarray(inputs["guide"], dtype=np.float32)
    inp = np.ascontiguousarray(inputs["input"], dtype=np.float32)
    B = guide.shape[0]
    assert guide.shape == (8, 3, H, W) and inp.shape == (8, 3, H, W)
    nc = _get_nc()
    bnd, eye, ney, lum = _get_consts()
    in_maps = [
        {"guide": guide[i], "input": inp[i], "bnd": bnd, "eye": eye,
         "ney": ney, "lum": lum}
        for i in range(B)
    ]
    res = bass_utils.run_bass_kernel_spmd(nc, in_maps, core_ids=list(range(B)))
    return np.stack([res.results[i]["out"] for i in range(B)], axis=0)


# revision 19
# speedup vs baseline: 1.2888x; 1.2888x over previous
"""Guided filter, 8 TRN2 cores, 1 image/core — all-PE box filters.

Each 31x31 box = two PE "Type-2" passes (data stationary in bf16, bf16 0/1
band matrix moving): out[m,n] = sum_k src[k,m]*band[k,n] box-filters along
the partition dim AND transposes, so two passes restore orientation. Band
sparsity -> 4 overlapping column-range pieces per output PSUM bank (zero-
region semantics permit overlapping accumulation after one start=True).
A PSUM->SBUF bf16 "mid" copy sits between the passes (PE reads SBUF only).

Stats are uncentered; the var/cov mean corrections (vs = sg^2, u2 =
mpp*sg) are computed in f32 and subtracted IN PSUM via -I @ x f32r
identity matmuls joining the pass-2 accumulation group. bf16 lives only
on box inputs/mids where rounding is random and averages out.

  s1 = g0*(WR/WB)+g2 ; s2 = g1*(WG/WB)+s1  (= gray/WB, kept for w)
  G = s2*WB ; G2 = G*G ; P_c = p_c ; GP_c = G*P_c            [bf16]
  ps_G = box(G) ; sg = ps_G/31
  ps_G2 = box(G2) - I@(sg^2)        -> rr = 1/ps_G2 (= 1/(961 var))
  ps_P = box(P_c) ; mpp = ps_P/31
  ps_GP = box(GP_c) - I@(mpp*sg)    -> covR in PSUM
  a = ps_GP*rr ; vpr = ps_GP*(rr*sg) ; bR = mpp-vpr          [bf16]
  ps_B = box(bR) + I@wq ; wq = (box(a)*K2*WB*29791)*s2
  out = ps_B * K2/31
"""
import sys

sys.path.insert(0, "/opt/trn_rl_repo")

import numpy as np
import ml_dtypes
import concourse.bass as bass
import concourse.bacc as bacc
import concourse.mybir as mybir
import concourse.tile as tile
from concourse import bass_utils
from contextlib import ExitStack

F32 = mybir.dt.float32
F32R = mybir.dt.float32r
BF16 = mybir.dt.bfloat16
ALU = mybir.AluOpType
ACT = mybir.ActivationFunctionType

R = 15
H = W = 512
NB = 4
WR, WG, WB = 0.299, 0.587, 0.114
S31 = 1.0 / 31.0
K2 = 1.0 / 961.0
FUSE = 961.0 * 31.0


def band_np():
    """BND[p, b, n] = 1 if |128b+p - n| <= R else 0."""
    k = np.arange(512)
    B = (np.abs(k[:, None] - k[None, :]) <= R).astype(np.float32)
    return B.reshape(4, 128, 512).transpose(1, 0, 2).copy()


def _build(nc):
    guide_d = nc.dram_tensor("guide", [3, H, W], F32R, kind="ExternalInput").ap()
    lum_d = nc.dram_tensor("lum", [128, 3, 128], F32R, kind="ExternalInput").ap()
    input_d = nc.dram_tensor("input", [3, H, W], F32, kind="ExternalInput").ap()
    bnd_d = nc.dram_tensor("bnd", [128, NB, W], BF16, kind="ExternalInput").ap()
    eye_d = nc.dram_tensor("eye", [128, 128], BF16, kind="ExternalInput").ap()
    ney_d = nc.dram_tensor("ney", [128, 128], F32R, kind="ExternalInput").ap()
    out_d = nc.dram_tensor("out", [3, H, W], F32, kind="ExternalOutput").ap()

    def plane(dram, c):
        return dram[c].rearrange("(b p) w -> p b w", p=128)

    def plane_half(dram, c, h):
        return dram[c].rearrange("(b p) w -> p b w", p=128)[:, 2 * h:2 * h + 2, :]

    with tile.TileContext(nc) as tc, ExitStack() as ctx:
        cons = ctx.enter_context(tc.tile_pool(name="cons", bufs=1))
        gpool = ctx.enter_context(tc.tile_pool(name="gpool", bufs=1))
        ppool = ctx.enter_context(tc.tile_pool(name="ppool", bufs=1))
        bfp = ctx.enter_context(tc.tile_pool(name="bfp", bufs=1))
        midp = ctx.enter_context(tc.tile_pool(name="midp", bufs=5))
        stat = ctx.enter_context(tc.tile_pool(name="stat", bufs=1))
        ps1p = ctx.enter_context(tc.tile_pool(name="ps1p", bufs=2, space="PSUM"))
        ps2p = ctx.enter_context(tc.tile_pool(name="ps2p", bufs=2, space="PSUM"))

        BND = cons.tile([128, NB, W], BF16, tag="BND", name="BND")
        nc.sync.dma_start(BND[:], bnd_d)
        EYE = cons.tile([128, 128], BF16, tag="EYE", name="EYE")
        nc.sync.dma_start(EYE[:], eye_d)
        NEY = cons.tile([128, 128], F32R, tag="NEY", name="NEY")
        nc.sync.dma_start(NEY[:], ney_d)

        g = [gpool.tile([128, NB, W], F32R, tag=f"g{c}", name=f"g{c}")
             for c in range(3)]
        LUM = cons.tile([128, 3, 128], F32R, tag="LUM", name="LUM")
        nc.sync.dma_start(LUM[:], lum_d)
        p = [ppool.tile([128, NB, W], F32, tag=f"p{c}", name=f"p{c}")
             for c in range(3)]
        # guide planes in halves; p0 early so box(P0) leads the PE stream
        for c in (0, 1, 2):
            nc.sync.dma_start(g[c][:, 0:2, :], plane_half(guide_d, c, 0))
        nc.sync.dma_start(p[0][:], plane(input_d, 0))
        for c in (0, 1, 2):
            nc.sync.dma_start(g[c][:, 2:4, :], plane_half(guide_d, c, 1))
        nc.sync.dma_start(p[1][:], plane(input_d, 1))
        nc.sync.dma_start(p[2][:], plane(input_d, 2))

        PCS = []
        for ki in range(NB):
            PCS.append((ki, max(0, 128 * ki - R), min(512, 128 * ki + 128 + R)))

        def pass1_half(dst_ps, src, mh):
            """cols half mh: out col-blocks {2mh, 2mh+1} (2 banks)"""
            for kgrp in ((0, 1), (2, 3)):
                for j in range(2):
                    mi = 2 * mh + j
                    for ki in kgrp:
                        _, n0, n1 = PCS[ki]
                        nc.tensor.matmul(
                            dst_ps[:, j, n0:n1],
                            src[:, ki, 128 * mi:128 * mi + 128],
                            BND[:, ki, n0:n1],
                            start=(ki == 0), stop=(ki == NB - 1))

        def pass2_half(dst, ymid, h, fuse):
            """fuse: None | ('bf16', rhs_tile) | ('f32r', rhs_tile)"""
            for j in range(2):
                mi = 2 * h + j
                for ki in range(NB):
                    _, n0, n1 = PCS[ki]
                    nc.tensor.matmul(
                        dst[:, j, n0:n1],
                        ymid[:, ki, 128 * mi:128 * mi + 128],
                        BND[:, ki, n0:n1],
                        start=(ki == 0),
                        stop=(ki == NB - 1 and fuse is None))
                if fuse is not None:
                    kind, rhs = fuse
                    if kind == "bf16":
                        nc.tensor.matmul(
                            dst[:, j, :], EYE[:], rhs[:, mi, :],
                            start=False, stop=True)
                    else:
                        nc.tensor.matmul(
                            dst[:, j, :], NEY[:], rhs[:, mi, :],
                            start=False, stop=True)

        def box(src, mid_engine, name, fuse=None):
            """[ps_half0, ps_half1] raw 31x31 box sums of src (+fused adds)."""
            ymid = midp.tile([128, NB, W], BF16, tag="mid", name=f"mid_{name}")
            for mh in range(2):
                ps1 = ps1p.tile([128, 2, W], F32, tag="ps1",
                                name=f"ps1_{name}{mh}")
                pass1_half(ps1, src, mh)
                if mid_engine == "act":
                    nc.scalar.copy(ymid[:, 2 * mh:2 * mh + 2, :], ps1[:])
                else:
                    nc.vector.tensor_copy(ymid[:, 2 * mh:2 * mh + 2, :],
                                          ps1[:])
            halves = []
            for h in range(2):
                ph = ps2p.tile([128, 2, W], F32, tag="ps2", name=f"ps2_{name}{h}")
                pass2_half(ph, ymid, h, fuse)
                halves.append(ph)
            return halves

        HV = [slice(0, 2), slice(2, 4)]

        # ---- luma on PE: ps_lum[b] = sum_c (w_c/WB * I) @ g_c[b] ----
        G = bfp.tile([128, NB, W], BF16, tag="G", name="G")
        for h in range(2):
            ps_lum = ps2p.tile([128, 2, W], F32, tag="ps2", name=f"ps_lum{h}")
            for j in range(2):
                b = 2 * h + j
                for ci in range(3):
                    nc.tensor.matmul(ps_lum[:, j, :], LUM[:, ci, :],
                                     g[ci][:, b, :], start=(ci == 0),
                                     stop=(ci == 2))
            if h == 0:
                nc.scalar.copy(G[:, 0:2, :], ps_lum[:])
            else:
                nc.vector.tensor_copy(G[:, 2:4, :], ps_lum[:])
        G2 = bfp.tile([128, NB, W], BF16, tag="G2", name="G2")
        for h in range(2):
            nc.gpsimd.tensor_mul(G2[:, HV[h], :], G[:, HV[h], :],
                                 G[:, HV[h], :])

        Pb = [None, None, None]
        mpp = [None, None, None]

        def p_cluster(c, mid_engine):
            Pb[c] = bfp.tile([128, NB, W], BF16, tag=f"Pb{c}", name=f"Pb{c}")
            nc.vector.tensor_copy(Pb[c][:], p[c][:])
            hP = box(Pb[c], mid_engine, f"P{c}")
            mpp[c] = stat.tile([128, NB, W], F32, tag=f"mpp{c}",
                               name=f"mpp{c}")
            for h in range(2):
                nc.scalar.mul(mpp[c][:, HV[h], :], hP[h][:], S31)

        p_cluster(0, "dve")

        # ---- box(G): sg + vs (vs = (ps*S31)^2 straight from PSUM, f32r)
        hG = box(G, "act", "G")
        sg = stat.tile([128, NB, W], F32, tag="sg", name="sg")
        vs = stat.tile([128, NB, W], F32R, tag="vs", name="vs")
        for h in range(2):
            nc.scalar.mul(sg[:, HV[h], :], hG[h][:], S31)
            nc.scalar.activation(vs[:, HV[h], :], hG[h][:], ACT.Square,
                                 scale=S31)
        sg_bf = stat.tile([128, NB, W], BF16, tag="sg_bf", name="sg_bf")
        nc.vector.tensor_copy(sg_bf[:], sg[:])

        p_cluster(1, "act")

        # ---- box(G2) - I@vs -> rr ----
        hG2 = box(G2, "dve", "G2", fuse=("f32r", vs))
        rr = stat.tile([128, NB, W], F32, tag="rr", name="rr")
        for h in range(2):
            nc.vector.reciprocal_approx_fast(rr[:, HV[h], :], hG2[h][:])
        rr_bf = stat.tile([128, NB, W], BF16, tag="rr_bf", name="rr_bf")
        nc.vector.tensor_copy(rr_bf[:], rr[:])
        t1 = stat.tile([128, NB, W], BF16, tag="t1", name="t1")
        nc.vector.tensor_tensor(t1[:], rr_bf[:], sg_bf[:], ALU.mult)

        # ---- per-channel covariance chains ----
        cv = [None, None, None]
        a_t = [None, None, None]
        bR = [None, None, None]

        def gp_cluster(c, mid_e):
            GPt = bfp.tile([128, NB, W], BF16, tag=f"GP{c}", name=f"GP{c}")
            nc.gpsimd.tensor_mul(GPt[:], G[:], Pb[c][:])
            u2 = stat.tile([128, NB, W], F32R, tag="scr", name=f"u2{c}")
            for h in range(2):
                nc.gpsimd.tensor_mul(u2[:, HV[h], :], mpp[c][:, HV[h], :],
                                     sg[:, HV[h], :])
            hGP = box(GPt, mid_e, f"GP{c}", fuse=("f32r", u2))
            cv[c] = stat.tile([128, NB, W], BF16, tag=f"cv{c}", name=f"cv{c}")
            for h in range(2):
                nc.scalar.copy(cv[c][:, HV[h], :], hGP[h][:])
            a_t[c] = bfp.tile([128, NB, W], BF16, tag=f"Pb{c}", name=f"a{c}")
            nc.vector.tensor_tensor(a_t[c][:], cv[c][:], rr_bf[:], ALU.mult)
            vpr = bfp.tile([128, NB, W], BF16, tag=f"GP{c}", name=f"vpr{c}")
            nc.vector.tensor_tensor(vpr[:], cv[c][:], t1[:], ALU.mult)
            bR[c] = stat.tile([128, NB, W], BF16, tag=f"bR{c}", name=f"bR{c}")
            nc.vector.tensor_tensor(bR[c][:], mpp[c][:], vpr[:], ALU.subtract)

        p_cluster(2, "act")
        gp_cluster(0, "dve")
        gp_cluster(1, "act")
        gp_cluster(2, "dve")

        # ---- output boxes, staggered A0 A1 B0 A2 B1 B2 ----
        wq = [None, None, None]

        def a_cluster(c, mid_e):
            hA = box(a_t[c], mid_e, f"A{c}")
            wq[c] = gpool.tile([128, NB, W], BF16, tag=f"g{c}", name=f"wq{c}")
            for h in range(2):
                nc.vector.scalar_tensor_tensor(
                    wq[c][:, HV[h], :], hA[h][:], 31.0,
                    G[:, HV[h], :], ALU.mult, ALU.mult)

        def b_cluster(c, mid_e):
            hB = box(bR[c], mid_e, f"B{c}", fuse=("bf16", wq[c]))
            ot = ppool.tile([128, NB, W], F32, tag=f"p{c}", name=f"ot{c}")
            for h in range(2):
                nc.scalar.mul(ot[:, HV[h], :], hB[h][:], K2 * S31)
                nc.sync.dma_start(plane_half(out_d, c, h),
                                  ot[:, HV[h], :])

        a_cluster(0, "act")
        a_cluster(1, "dve")
        b_cluster(0, "act")
        a_cluster(2, "dve")
        b_cluster(1, "act")
        b_cluster(2, "dve")

    nc.compile()
    return nc


_NC_CACHE = None


def _get_nc():
    global _NC_CACHE
    if _NC_CACHE is None:
        nc = bacc.Bacc("TRN2", target_bir_lowering=False, debug=False)
        _build(nc)
        _NC_CACHE = nc
    return _NC_CACHE


_CONSTS = None


def _get_consts():
    global _CONSTS
    if _CONSTS is None:
        lum = np.stack([np.eye(128, dtype=np.float32) * (w / WB)
                        for w in (WR, WG, WB)])   # [3,128,128]
        lum = lum.transpose(1, 0, 2).copy()       # [128,3,128] lhsT layout
        _CONSTS = (band_np().astype(ml_dtypes.bfloat16),
                   np.eye(128, dtype=np.float32).astype(ml_dtypes.bfloat16),
                   (-np.eye(128, dtype=np.float32)), lum)
    return _CONSTS


def kernel(**inputs):
    guide = np.ascontiguousarray(inputs["guide"], dtype=np.float32)
    inp = np.ascontiguousarray(inputs["input"], dtype=np.float32)
    B = guide.shape[0]
    assert guide.shape == (8, 3, H, W) and inp.shape == (8, 3, H, W)
    nc = _get_nc()
    bnd, eye, ney, lum = _get_consts()
    in_maps = [
        {"guide": guide[i], "input": inp[i], "bnd": bnd, "eye": eye,
         "ney": ney, "lum": lum}
        for i in range(B)
    ]
    res = bass_utils.run_bass_kernel_spmd(nc, in_maps, core_ids=list(range(B)))
    return np.stack([res.results[i]["out"] for i in range(B)], axis=0)
